# revision 17
# baseline (speedup 1.0000x reference)
"""Trainium2 Bass kernel for channel-wise weighted reduction + capped relu.

Computes out[b, s] = capped_relu(sum_c x[b,c,s] * W[c,s] + bias[s]) for
x [64, 256, 4096] f32, W [256, 4096] f32, bias [4096] f32.

Sharding: data-parallel over batch across 8 NeuronCores (8 batches/core),
weights + bias replicated. No cross-core communication.

Per-core pipeline:
  - DMA x[b] as one SBUF tile [128ch, 2*4096] (two 2 MiB transfers).
  - DVE: y = x * W elementwise (in-place), one [128, 4096] op per c-half.
  - PE:  channel reduction as matmul with ones[128,1] STATIONARY (loaded
    once, 1 column) and the products MOVING: out row psum[b, chunk] =
    ones.T @ y_chunk. fp32 moving rows cost 4 cyc/row; for FOLD_BATCHES
    of the 8 batches the two c-halves are pre-summed on DVE (one extra
    [128,4096] add) which halves that batch's PE stream — the knob
    balances DVE vs PE occupancy.
  - Epilogue on [8, 4096]: tb = psum + bias ; mask = is_le(max(tb,0),1) ;
    o = max(tb,0)*mask ; direct row-major store.
"""

import numpy as np

B, C, S = 64, 256, 4096
NCORES = 8
BPC = B // NCORES          # batches per core
NJ = S // 512              # 8 psum-bank chunks of 512
H = C // 128               # 2 channel halves

_cache = {}


def _build_nc(fold_batches=4, use_f32r=False):
    import concourse.bacc as bacc
    import concourse.bass as bass
    import concourse.mybir as mybir
    from concourse.tile import TileContext

    f32 = mybir.dt.float32
    Alu = mybir.AluOpType

    nc = bacc.Bacc(
        "TRN2",
        target_bir_lowering=False,
        debug=False,
        num_devices=NCORES,
    )

    x_d = nc.dram_tensor("x", [BPC, C, S], f32, kind="ExternalInput").ap()
    w_d = nc.dram_tensor("weights", [C, S], f32, kind="ExternalInput").ap()
    b_d = nc.dram_tensor("bias", [S], f32, kind="ExternalInput").ap()
    o_d = nc.dram_tensor("out", [BPC, S], f32, kind="ExternalOutput").ap()

    with TileContext(nc) as tc:
        NQ = 4                  # s-quarters per c-half for DMA/compute chunking
        QS = S // NQ
        with (
            tc.tile_pool(name="consts", bufs=1) as cpool,
            tc.tile_pool(name="xbuf", bufs=3) as xpool,
            tc.tile_pool(name="stg", bufs=1) as spool,
            tc.tile_pool(name="epi", bufs=1) as epool,
            tc.tile_pool(name="ps", bufs=1, space="PSUM") as ppool,
        ):
            # Replicated weights, both halves side by side: [:, h*S:(h+1)*S].
            # W loads are emitted per (h, q) chunk, interleaved with batch 0's
            # x chunks below, so the first multiply starts after ~2 MiB of
            # DMA instead of waiting for all of W.
            w_t = cpool.tile([128, H * S], f32, name="w_t")

            ones_t = cpool.tile([128, 1], f32, name="ones_t")
            nc.vector.memset(ones_t[:], 1.0)

            # bias broadcast to the 8 output rows
            bias_bc = cpool.tile([BPC, S], f32, name="bias_bc")

            # PE output rows must sit on 32-aligned partitions, and a PSUM
            # bank being read (ACT drain) while the PE writes it serializes
            # the pipeline. Slot map: batch parity picks the bank half
            # (free-dim half), (b//2)%2 picks the row pair — consecutive
            # batches touch disjoint banks, so drains overlap next batch's
            # matmuls. Each batch's 4096-wide row lives as 2 half-rows:
            #   chunk j -> row 32*(2*((b//2)%2) + j//4),
            #             free offset (S//2)*(b%2) + (j%4)*512.
            psum_big = ppool.tile([128, S], f32, name="psum_big")
            out_acc = epool.tile([BPC, S], f32, name="out_acc")

            def chunk(base, h, q):
                return slice(base + h * S + q * QS, base + h * S + (q + 1) * QS)

            for b in range(BPC):
                hb = b % 2              # bank half (free-dim half)
                rp = (b // 2) % 2       # row pair
                x_t = xpool.tile([128, H * S], f32, name="x_t", tag="x")
                for h in range(H):
                    for q in range(NQ):
                        if b == 0:
                            nc.sync.dma_start(
                                w_t[:, chunk(0, h, q)],
                                w_d[h * 128:(h + 1) * 128, q * QS:(q + 1) * QS],
                            )
                        nc.sync.dma_start(
                            x_t[:, chunk(0, h, q)],
                            x_d[b, h * 128:(h + 1) * 128, q * QS:(q + 1) * QS],
                        )
                if b == 0:
                    for bb in range(BPC):
                        nc.sync.dma_start(bias_bc[bb:bb + 1, :], b_d[None, :])
                fold = b < fold_batches
                nhalf = 1 if fold else H
                for q in range(NQ):
                    for h in range(H):
                        nc.vector.tensor_tensor(
                            x_t[:, chunk(0, h, q)],
                            x_t[:, chunk(0, h, q)],
                            w_t[:, chunk(0, h, q)],
                            Alu.mult,
                        )
                    if fold:
                        # z = y_h0 + y_h1 in place -> halves the PE stream
                        nc.vector.tensor_tensor(
                            x_t[:, chunk(0, 0, q)],
                            x_t[:, chunk(0, 0, q)],
                            x_t[:, chunk(0, 1, q)],
                            Alu.add,
                        )
                    for j in (2 * q, 2 * q + 1):
                        row = 32 * (2 * rp + j // 4)
                        off = (S // 2) * hb + (j % 4) * 512
                        for h in range(nhalf):
                            rhs = x_t[:, h * S + j * 512: h * S + (j + 1) * 512]
                            lhsT = ones_t[:, 0:1]
                            if use_f32r:
                                rhs = rhs.bitcast(mybir.dt.float32r)
                                lhsT = lhsT.bitcast(mybir.dt.float32r)
                            nc.tensor.matmul(
                                psum_big[row:row + 1, off:off + 512],
                                lhsT,
                                rhs,
                                start=(h == 0),
                                stop=(h == nhalf - 1),
                                tile_position=(0, row),
                            )
                # Drain this batch's two half-rows: compute engines can only
                # address 32-aligned SBUF partition windows, so ACT-copy each
                # psum half-row to a partition-0 staging row, then pack it
                # onto partition b of out_acc with an SBUF->SBUF DMA
                # (DMA has no partition-alignment restriction).
                stg = spool.tile([1, S], f32, name="stg", tag="stg")
                for half in range(2):
                    row = 32 * (2 * rp + half)
                    off = (S // 2) * hb
                    nc.scalar.activation(
                        stg[:, half * (S // 2):(half + 1) * (S // 2)],
                        psum_big[row:row + 1, off:off + S // 2],
                        mybir.ActivationFunctionType.Copy,
                    )
                nc.sync.dma_start(out_acc[b:b + 1, :], stg[:, :])

            # Epilogue: capped relu on [8, 4096] in two s-halves, computed
            # in place on out_acc, then row-major store.
            for s0 in (0, S // 2):
                sl = slice(s0, s0 + S // 2)
                nc.vector.tensor_tensor(
                    out_acc[:, sl], out_acc[:, sl], bias_bc[:, sl], Alu.add
                )
                msk = epool.tile([BPC, S // 2], f32, name="msk", tag="msk", bufs=1)
                nc.vector.tensor_scalar(msk[:], out_acc[:, sl], 0.0, 1.0, Alu.max, Alu.is_le)
                nc.vector.scalar_tensor_tensor(
                    out_acc[:, sl], out_acc[:, sl], 0.0, msk[:], Alu.max, Alu.mult
                )
                nc.sync.dma_start(o_d[:, sl], out_acc[:, sl])

    nc.compile()
    return nc


def kernel(x: np.ndarray, weights: np.ndarray, bias: np.ndarray) -> np.ndarray:
    from concourse.bass_utils import run_bass_kernel_spmd

    if "nc" not in _cache:
        _cache["nc"] = _build_nc()
    nc = _cache["nc"]

    x = np.ascontiguousarray(x, dtype=np.float32)
    weights = np.ascontiguousarray(weights, dtype=np.float32)
    bias = np.ascontiguousarray(bias, dtype=np.float32)

    in_maps = [
        {
            "x": x[i * BPC:(i + 1) * BPC],
            "weights": weights,
            "bias": bias,
        }
        for i in range(NCORES)
    ]
    res = run_bass_kernel_spmd(nc, in_maps, core_ids=list(range(NCORES)))
    return np.concatenate([res.results[i]["out"] for i in range(NCORES)], axis=0)


# revision 22
# speedup vs baseline: 1.1586x; 1.1586x over previous
"""Trainium2 Bass kernel for channel-wise weighted reduction + capped relu.

Computes out[b, s] = capped_relu(sum_c x[b,c,s] * W[c,s] + bias[s]) for
x [64, 256, 4096] f32, W [256, 4096] f32, bias [4096] f32.

Sharding: data-parallel over batch across 8 NeuronCores (8 batches/core),
weights + bias replicated. No cross-core communication.

Per-core pipeline:
  - DMA x[b] as one SBUF tile [128ch, 2*4096] (two 2 MiB transfers).
  - DVE: y = x * W elementwise (in-place), one [128, 4096] op per c-half.
  - PE:  channel reduction as matmul with ones[128,1] STATIONARY (loaded
    once, 1 column) and the products MOVING: out row psum[b, chunk] =
    ones.T @ y_chunk. fp32 moving rows cost 4 cyc/row; for FOLD_BATCHES
    of the 8 batches the two c-halves are pre-summed on DVE (one extra
    [128,4096] add) which halves that batch's PE stream — the knob
    balances DVE vs PE occupancy.
  - Epilogue on [8, 4096]: tb = psum + bias ; mask = is_le(max(tb,0),1) ;
    o = max(tb,0)*mask ; direct row-major store.
"""

import numpy as np

B, C, S = 64, 256, 4096
NCORES = 8
BPC = B // NCORES          # batches per core
NJ = S // 512              # 8 psum-bank chunks of 512
H = C // 128               # 2 channel halves

_cache = {}


def _build_nc(fold_batches=4, use_f32r=False):
    import concourse.bacc as bacc
    import concourse.bass as bass
    import concourse.mybir as mybir
    from concourse.tile import TileContext

    f32 = mybir.dt.float32
    Alu = mybir.AluOpType

    nc = bacc.Bacc(
        "TRN2",
        target_bir_lowering=False,
        debug=False,
        num_devices=NCORES,
    )

    x_d = nc.dram_tensor("x", [BPC, C, S], f32, kind="ExternalInput").ap()
    w_d = nc.dram_tensor("weights", [C, S], f32, kind="ExternalInput").ap()
    b_d = nc.dram_tensor("bias", [S], f32, kind="ExternalInput").ap()
    o_d = nc.dram_tensor("out", [BPC, S], f32, kind="ExternalOutput").ap()

    with TileContext(nc) as tc:
        NQ = 4                  # s-quarters per c-half for DMA/compute chunking
        QS = S // NQ
        with (
            tc.tile_pool(name="consts", bufs=1) as cpool,
            tc.tile_pool(name="xbuf", bufs=3) as xpool,
            tc.tile_pool(name="stg", bufs=2) as spool,
            tc.tile_pool(name="epi", bufs=1) as epool,
            tc.tile_pool(name="ps", bufs=1, space="PSUM") as ppool,
        ):
            # Replicated weights, both halves side by side: [:, h*S:(h+1)*S].
            # W loads are emitted per (h, q) chunk, interleaved with batch 0's
            # x chunks below, so the first multiply starts after ~2 MiB of
            # DMA instead of waiting for all of W.
            w_t = cpool.tile([128, H * S], f32, name="w_t")

            ones_t = cpool.tile([128, 1], f32, name="ones_t")
            nc.vector.memset(ones_t[:], 1.0)

            # PE output rows must sit on 32-aligned partitions, and a PSUM
            # bank being read (ACT drain) while the PE writes it serializes
            # the pipeline. Slot map: batch parity picks the bank half
            # (free-dim half), (b//2)%2 picks the row pair — consecutive
            # batches touch disjoint banks, so drains overlap next batch's
            # matmuls. Each batch's 4096-wide row lives as 2 half-rows:
            #   chunk j -> row 32*(2*((b//2)%2) + j//4),
            #             free offset (S//2)*(b%2) + (j%4)*512.
            psum_big = ppool.tile([128, S], f32, name="psum_big")
            # out_acc is pre-loaded with bias; each batch's sums are packed
            # onto row b with an ACCUMULATING SWDGE DMA (out += stg), which
            # fuses the bias add for free. Small/late-bound DMAs (bias,
            # drain-pack, stores) go on the scalar/gpsimd queues so they
            # can't head-of-line-block the x prefetch stream on sync.
            out_acc = epool.tile([BPC, S], f32, name="out_acc")
            for bb in range(BPC):
                nc.scalar.dma_start(out_acc[bb:bb + 1, :], b_d[None, :])

            def chunk(base, h, q):
                return slice(base + h * S + q * QS, base + h * S + (q + 1) * QS)

            for b in range(BPC):
                hb = b % 2              # bank half (free-dim half)
                rp = (b // 2) % 2       # row pair
                x_t = xpool.tile([128, H * S], f32, name="x_t", tag="x")
                for h in range(H):
                    for q in range(NQ):
                        if b == 0:
                            nc.sync.dma_start(
                                w_t[:, chunk(0, h, q)],
                                w_d[h * 128:(h + 1) * 128, q * QS:(q + 1) * QS],
                            )
                        nc.sync.dma_start(
                            x_t[:, chunk(0, h, q)],
                            x_d[b, h * 128:(h + 1) * 128, q * QS:(q + 1) * QS],
                        )
                fold = b < fold_batches
                nhalf = 1 if fold else H
                for q in range(NQ):
                    for h in range(H):
                        nc.vector.tensor_tensor(
                            x_t[:, chunk(0, h, q)],
                            x_t[:, chunk(0, h, q)],
                            w_t[:, chunk(0, h, q)],
                            Alu.mult,
                        )
                    if fold:
                        # z = y_h0 + y_h1 in place -> halves the PE stream
                        nc.vector.tensor_tensor(
                            x_t[:, chunk(0, 0, q)],
                            x_t[:, chunk(0, 0, q)],
                            x_t[:, chunk(0, 1, q)],
                            Alu.add,
                        )
                    for j in (2 * q, 2 * q + 1):
                        row = 32 * (2 * rp + j // 4)
                        off = (S // 2) * hb + (j % 4) * 512
                        for h in range(nhalf):
                            rhs = x_t[:, h * S + j * 512: h * S + (j + 1) * 512]
                            lhsT = ones_t[:, 0:1]
                            if use_f32r:
                                rhs = rhs.bitcast(mybir.dt.float32r)
                                lhsT = lhsT.bitcast(mybir.dt.float32r)
                            nc.tensor.matmul(
                                psum_big[row:row + 1, off:off + 512],
                                lhsT,
                                rhs,
                                start=(h == 0),
                                stop=(h == nhalf - 1),
                                tile_position=(0, row),
                            )
                # Drain this batch's two half-rows: compute engines can only
                # address 32-aligned SBUF partition windows, so ACT-copy each
                # psum half-row to a partition-0 staging row, then pack it
                # onto partition b of out_acc with an SBUF->SBUF DMA
                # (DMA has no partition-alignment restriction).
                stg = spool.tile([1, S], f32, name="stg", tag="stg")
                for half in range(2):
                    row = 32 * (2 * rp + half)
                    off = (S // 2) * hb
                    nc.scalar.activation(
                        stg[:, half * (S // 2):(half + 1) * (S // 2)],
                        psum_big[row:row + 1, off:off + S // 2],
                        mybir.ActivationFunctionType.Copy,
                    )
                nc.gpsimd.dma_start(
                    out_acc[b:b + 1, :], stg[:, :], accum_op=Alu.add
                )

            # Epilogue: capped relu on [8, 4096] in two s-halves, computed
            # in place on out_acc (bias already folded in by the accumulating
            # pack DMAs), then row-major store.
            for s0 in (0, S // 2):
                sl = slice(s0, s0 + S // 2)
                msk = epool.tile([BPC, S // 2], f32, name="msk", tag="msk", bufs=1)
                nc.vector.tensor_scalar(msk[:], out_acc[:, sl], 0.0, 1.0, Alu.max, Alu.is_le)
                nc.vector.scalar_tensor_tensor(
                    out_acc[:, sl], out_acc[:, sl], 0.0, msk[:], Alu.max, Alu.mult
                )
                nc.scalar.dma_start(o_d[:, sl], out_acc[:, sl])

    nc.compile()
    return nc


def kernel(x: np.ndarray, weights: np.ndarray, bias: np.ndarray) -> np.ndarray:
    from concourse.bass_utils import run_bass_kernel_spmd

    if "nc" not in _cache:
        _cache["nc"] = _build_nc()
    nc = _cache["nc"]

    x = np.ascontiguousarray(x, dtype=np.float32)
    weights = np.ascontiguousarray(weights, dtype=np.float32)
    bias = np.ascontiguousarray(bias, dtype=np.float32)

    in_maps = [
        {
            "x": x[i * BPC:(i + 1) * BPC],
            "weights": weights,
            "bias": bias,
        }
        for i in range(NCORES)
    ]
    res = run_bass_kernel_spmd(nc, in_maps, core_ids=list(range(NCORES)))
    return np.concatenate([res.results[i]["out"] for i in range(NCORES)], axis=0)


# revision 24
# speedup vs baseline: 1.2582x; 1.0859x over previous
"""Trainium2 Bass kernel for channel-wise weighted reduction + capped relu.

Computes out[b, s] = capped_relu(sum_c x[b,c,s] * W[c,s] + bias[s]) for
x [64, 256, 4096] f32, W [256, 4096] f32, bias [4096] f32.

Sharding: data-parallel over batch across 8 NeuronCores (8 batches/core),
weights + bias replicated. No cross-core communication.

Per-core pipeline:
  - DMA x[b] as one SBUF tile [128ch, 2*4096] (two 2 MiB transfers).
  - DVE: y = x * W elementwise (in-place), one [128, 4096] op per c-half.
  - PE:  channel reduction as matmul with ones[128,1] STATIONARY (loaded
    once, 1 column) and the products MOVING: out row psum[b, chunk] =
    ones.T @ y_chunk. fp32 moving rows cost 4 cyc/row; for FOLD_BATCHES
    of the 8 batches the two c-halves are pre-summed on DVE (one extra
    [128,4096] add) which halves that batch's PE stream — the knob
    balances DVE vs PE occupancy.
  - Epilogue on [8, 4096]: tb = psum + bias ; mask = is_le(max(tb,0),1) ;
    o = max(tb,0)*mask ; direct row-major store.
"""

import numpy as np

B, C, S = 64, 256, 4096
NCORES = 8
BPC = B // NCORES          # batches per core
NJ = S // 512              # 8 psum-bank chunks of 512
H = C // 128               # 2 channel halves

_cache = {}


def _build_nc(fold_batches=2, use_f32r=False):
    import concourse.bacc as bacc
    import concourse.bass as bass
    import concourse.mybir as mybir
    from concourse.tile import TileContext

    f32 = mybir.dt.float32
    Alu = mybir.AluOpType

    nc = bacc.Bacc(
        "TRN2",
        target_bir_lowering=False,
        debug=False,
        num_devices=NCORES,
    )

    x_d = nc.dram_tensor("x", [BPC, C, S], f32, kind="ExternalInput").ap()
    w_d = nc.dram_tensor("weights", [C, S], f32, kind="ExternalInput").ap()
    b_d = nc.dram_tensor("bias", [S], f32, kind="ExternalInput").ap()
    o_d = nc.dram_tensor("out", [BPC, S], f32, kind="ExternalOutput").ap()

    with TileContext(nc) as tc:
        NQ = 4                  # s-quarters per c-half for DMA/compute chunking
        QS = S // NQ
        with (
            tc.tile_pool(name="consts", bufs=1) as cpool,
            tc.tile_pool(name="xbuf", bufs=3) as xpool,
            tc.tile_pool(name="stg", bufs=2) as spool,
            tc.tile_pool(name="epi", bufs=1) as epool,
            tc.tile_pool(name="ps", bufs=1, space="PSUM") as ppool,
        ):
            # Replicated weights, both halves side by side: [:, h*S:(h+1)*S].
            # W loads are emitted per (h, q) chunk, interleaved with batch 0's
            # x chunks below, so the first multiply starts after ~2 MiB of
            # DMA instead of waiting for all of W.
            w_t = cpool.tile([128, H * S], f32, name="w_t")

            ones_t = cpool.tile([128, 1], f32, name="ones_t")
            nc.vector.memset(ones_t[:], 1.0)

            # PE output rows must sit on 32-aligned partitions, and a PSUM
            # bank being read (ACT drain) while the PE writes it serializes
            # the pipeline. Slot map: batch parity picks the bank half
            # (free-dim half), (b//2)%2 picks the row pair — consecutive
            # batches touch disjoint banks, so drains overlap next batch's
            # matmuls. Each batch's 4096-wide row lives as 2 half-rows:
            #   chunk j -> row 32*(2*((b//2)%2) + j//4),
            #             free offset (S//2)*(b%2) + (j%4)*512.
            psum_big = ppool.tile([128, S], f32, name="psum_big")
            # out_acc is pre-loaded with bias; each batch's sums are packed
            # onto row b with an ACCUMULATING SWDGE DMA (out += stg), which
            # fuses the bias add for free. Small/late-bound DMAs (bias,
            # drain-pack, stores) go on the scalar/gpsimd queues so they
            # can't head-of-line-block the x prefetch stream on sync.
            out_acc = epool.tile([BPC, S], f32, name="out_acc")
            for bb in range(BPC):
                nc.scalar.dma_start(out_acc[bb:bb + 1, :], b_d[None, :])

            def chunk(base, h, q):
                return slice(base + h * S + q * QS, base + h * S + (q + 1) * QS)

            for b in range(BPC):
                hb = b % 2              # bank half (free-dim half)
                rp = (b // 2) % 2       # row pair
                x_t = xpool.tile([128, H * S], f32, name="x_t", tag="x")
                # 2 MiB DMA transfers (best bandwidth); DVE still computes in
                # QS-wide chunks for pipelining.
                for h in range(H):
                    for dq in range(2):
                        lo, hi = dq * (S // 2), (dq + 1) * (S // 2)
                        if b == 0:
                            nc.sync.dma_start(
                                w_t[:, h * S + lo:h * S + hi],
                                w_d[h * 128:(h + 1) * 128, lo:hi],
                            )
                        nc.sync.dma_start(
                            x_t[:, h * S + lo:h * S + hi],
                            x_d[b, h * 128:(h + 1) * 128, lo:hi],
                        )
                fold = b < fold_batches
                nhalf = 1 if fold else H
                for q in range(NQ):
                    for h in range(H):
                        nc.vector.tensor_tensor(
                            x_t[:, chunk(0, h, q)],
                            x_t[:, chunk(0, h, q)],
                            w_t[:, chunk(0, h, q)],
                            Alu.mult,
                        )
                    if fold:
                        # z = y_h0 + y_h1 in place -> halves the PE stream
                        nc.vector.tensor_tensor(
                            x_t[:, chunk(0, 0, q)],
                            x_t[:, chunk(0, 0, q)],
                            x_t[:, chunk(0, 1, q)],
                            Alu.add,
                        )
                    for j in (2 * q, 2 * q + 1):
                        row = 32 * (2 * rp + j // 4)
                        off = (S // 2) * hb + (j % 4) * 512
                        for h in range(nhalf):
                            rhs = x_t[:, h * S + j * 512: h * S + (j + 1) * 512]
                            lhsT = ones_t[:, 0:1]
                            if use_f32r:
                                rhs = rhs.bitcast(mybir.dt.float32r)
                                lhsT = lhsT.bitcast(mybir.dt.float32r)
                            nc.tensor.matmul(
                                psum_big[row:row + 1, off:off + 512],
                                lhsT,
                                rhs,
                                start=(h == 0),
                                stop=(h == nhalf - 1),
                                tile_position=(0, row),
                            )
                # Drain this batch's two half-rows: compute engines can only
                # address 32-aligned SBUF partition windows, so ACT-copy each
                # psum half-row to a partition-0 staging row, then pack it
                # onto partition b of out_acc with an SBUF->SBUF DMA
                # (DMA has no partition-alignment restriction).
                stg = spool.tile([1, S], f32, name="stg", tag="stg")
                for half in range(2):
                    row = 32 * (2 * rp + half)
                    off = (S // 2) * hb
                    nc.scalar.activation(
                        stg[:, half * (S // 2):(half + 1) * (S // 2)],
                        psum_big[row:row + 1, off:off + S // 2],
                        mybir.ActivationFunctionType.Copy,
                    )
                nc.gpsimd.dma_start(
                    out_acc[b:b + 1, :], stg[:, :], accum_op=Alu.add
                )

            # Epilogue: capped relu on [8, 4096] in two s-halves, computed
            # in place on out_acc (bias already folded in by the accumulating
            # pack DMAs), then row-major store.
            for s0 in (0, S // 2):
                sl = slice(s0, s0 + S // 2)
                msk = epool.tile([BPC, S // 2], f32, name="msk", tag="msk", bufs=1)
                nc.vector.tensor_scalar(msk[:], out_acc[:, sl], 0.0, 1.0, Alu.max, Alu.is_le)
                nc.vector.scalar_tensor_tensor(
                    out_acc[:, sl], out_acc[:, sl], 0.0, msk[:], Alu.max, Alu.mult
                )
                nc.scalar.dma_start(o_d[:, sl], out_acc[:, sl])

    nc.compile()
    return nc


def kernel(x: np.ndarray, weights: np.ndarray, bias: np.ndarray) -> np.ndarray:
    from concourse.bass_utils import run_bass_kernel_spmd

    if "nc" not in _cache:
        _cache["nc"] = _build_nc()
    nc = _cache["nc"]

    x = np.ascontiguousarray(x, dtype=np.float32)
    weights = np.ascontiguousarray(weights, dtype=np.float32)
    bias = np.ascontiguousarray(bias, dtype=np.float32)

    in_maps = [
        {
            "x": x[i * BPC:(i + 1) * BPC],
            "weights": weights,
            "bias": bias,
        }
        for i in range(NCORES)
    ]
    res = run_bass_kernel_spmd(nc, in_maps, core_ids=list(range(NCORES)))
    return np.concatenate([res.results[i]["out"] for i in range(NCORES)], axis=0)
